# revision 32
# baseline (speedup 1.0000x reference)
"""Fused dense_mlp kernel for TRN2 (8 NeuronCores, Bass/Tile).

reference math:
    y = x @ W.T + bias               # [B, OUT]
    pooled = avgpool_k4(y)           # [B, OUT/4]
    out = max_j( 2 * gelu_tanh(pooled) )   # [B]

Algebraic restructuring (exact, up to fp rounding):
  * avg-pool commutes with the linear layer:
        pooled = x @ Wp.T + bias_p,  Wp = mean of each 4-row group of W
    -> the GEMM shrinks 4x to [B, K] @ [K, J], K=4096, J=2048.
  * gelu_tanh is quasiconvex (single minimum ~ -0.75), so
        max_j gelu(p_j) = max(gelu(row_max), gelu(row_min))
    -> only 2 gelu evaluations per row, after cheap row max/min reductions.
  * SCALE=2 cancels gelu's 0.5:  2*gelu(p) = p * (1 + tanh(c0*(p + c1*p^3))).
  * the j-max commutes with sharding j: each core reports its partial
    max over its j-range; the host combines with an elementwise max.

Distribution: 2D sharding - 4 batch shards x 2 j shards. Core (t*4+s)
handles rows [s*4096,(s+1)*4096) and pooled features [t*1024,(t+1)*1024).
Its Wp half (16.8 MB) is fully SBUF-resident, x streams through exactly
once (k-major, pre-transposed on host).

Matmuls run as float32r (TF32-like fast fp32 path, 1 row/cycle at N=512).
Each stationary x tile feeds 2 PSUM banks (the two 512-wide j tiles), so
LDWEIGHTS is amortized over 2 matmuls. Bias is accumulated into PSUM via
one extra matmul with a ones-row stationary operand.
"""

import os
import sys

for _p in ("/opt/trn_rl_repo",):
    if _p not in sys.path:
        sys.path.append(_p)

import numpy as np

import concourse.bass as bass
import concourse.mybir as mybir
import concourse.tile as tile
from concourse import bacc, bass_utils

# Problem shapes (hardcoded per contract).
B, IN, OUT = 16384, 4096, 8192
POOL_K = 4
J = OUT // POOL_K            # 2048 pooled features
N_CORES = 8
BS = 4                       # batch shards
JS = 2                       # j shards
BL = B // BS                 # 4096 batch rows per core
JL = J // JS                 # 1024 pooled features per core
P = 128                      # partitions
KO = IN // P                 # 32 k-subtiles
NB = BL // P                 # 32 b-tiles per core
JT = 512                     # j-tile width (one PSUM bank)
NJ = JL // JT                # 2 j-tiles per core
WP_CHUNKS = 8                # split the resident wp load to unblock early MMs

C0 = 0.7978845608            # sqrt(2/pi) as used by the reference
C1 = 0.044715

F32 = mybir.dt.float32
F32R = mybir.dt.float32r

_cached = None


def _build():
    nc = bacc.Bacc("TRN2", target_bir_lowering=False)
    xt = nc.dram_tensor("xt", [IN, BL], F32R, kind="ExternalInput")
    wp = nc.dram_tensor("wp", [IN, JL], F32R, kind="ExternalInput")
    brow = nc.dram_tensor("brow", [P, JL], F32, kind="ExternalInput")
    out = nc.dram_tensor("out", [NB, P], F32, kind="ExternalOutput")

    xt_r = xt.ap().rearrange("(ko ki) b -> ki ko b", ki=P)
    wp_r = wp.ap().rearrange("(ko ki) j -> ki ko j", ki=P)

    with tile.TileContext(nc) as tc:
        with (
            tc.tile_pool(name="wpp", bufs=1) as wp_pool,
            tc.tile_pool(name="xp", bufs=6) as x_pool,
            tc.tile_pool(name="cst", bufs=1) as const_pool,
            tc.tile_pool(name="red", bufs=1) as red_pool,
            tc.tile_pool(name="fin", bufs=1) as fin_pool,
            tc.tile_pool(name="psum", bufs=4, space="PSUM") as psum_pool,
        ):
            # Whole wp half resident, as separate chunk tiles so the PE can
            # start as soon as chunk 0 lands (precise per-chunk deps).
            # First chunks are small so the first matmul starts early.
            wp_sizes = [4, 4, 4, 4, 4, 4, 4, 4]
            assert sum(wp_sizes) == KO and len(wp_sizes) == WP_CHUNKS
            wp_starts = [sum(wp_sizes[:c]) for c in range(WP_CHUNKS)]
            wp_ts = [None] * WP_CHUNKS
            ko2chunk = []
            for c, sz in enumerate(wp_sizes):
                ko2chunk += [c] * sz

            def load_wp(c):
                wpc_t = wp_pool.tile(
                    [P, wp_sizes[c], JL], F32R, tag=f"wp{c}", name=f"wp{c}"
                )
                nc.sync.dma_start(
                    wpc_t[:], wp_r[:, wp_starts[c]:wp_starts[c] + wp_sizes[c], :]
                )
                wp_ts[c] = wpc_t

            mx = red_pool.tile([P, NB, NJ], F32)
            mn = red_pool.tile([P, NB, NJ], F32)

            BIG = 3.0e38

            KH = KO // 2

            def load_x_half(b, h):
                t = x_pool.tile([P, KH, P], F32R, tag="x", name=f"x_{b}h{h}")
                nc.sync.dma_start(
                    t[:], xt_r[:, h * KH:(h + 1) * KH, b * P:(b + 1) * P]
                )
                return t

            def load_x(b):
                # two half tiles: finer slot release -> deeper x prefetch
                return (load_x_half(b, 0), load_x_half(b, 1))

            def alloc_ps(b):
                return [psum_pool.tile([P, JT], F32, tag=f"ps{j}", name=f"ps{j}_{b}")
                        for j in range(NJ)]

            def mm(x_pair, pss, ko):
                c = ko2chunk[ko]
                x_t = x_pair[ko // KH]
                for j, ps in enumerate(pss):
                    nc.tensor.matmul(
                        ps[:], lhsT=x_t[:, ko % KH, :],
                        rhs=wp_ts[c][:, ko - wp_starts[c], j * JT:(j + 1) * JT],
                        start=(ko == 0), stop=(ko == KO - 1),
                    )

            def reduce_ps(b, pss):
                # bias-add in place (DVE), then row max / min reductions
                for j, ps in enumerate(pss):
                    nc.vector.scalar_tensor_tensor(
                        ps[:], ps[:], 0.0, brow_t[:, j * JT:(j + 1) * JT],
                        op0=mybir.AluOpType.bypass, op1=mybir.AluOpType.add,
                    )
                    nc.vector.tensor_reduce(
                        mx[:, b, j:j + 1], ps[:], axis=mybir.AxisListType.X,
                        op=mybir.AluOpType.max, opt_output=False,
                    )
                    nc.vector.tensor_reduce(
                        mn[:, b, j:j + 1], ps[:], axis=mybir.AxisListType.X,
                        op=mybir.AluOpType.min, opt_output=False,
                    )

            # Warmup group: first GA b-tiles run chunk-major so the PE has
            # work while the later wp chunks are still loading. DMA issue
            # order interleaves the first x tiles with the wp chunks so the
            # first matmul can start after ~4 MB of traffic, and brow (only
            # needed by the first reduce) goes last.
            GA = 3
            xh0 = [None] * GA
            xh1 = [None] * GA
            xh0[0] = load_x_half(0, 0)
            load_wp(0)
            xh0[1] = load_x_half(1, 0)
            load_wp(1)
            xh0[2] = load_x_half(2, 0)
            load_wp(2)
            load_wp(3)
            # second halves: first needed at chunk 4 (ko 16)
            for b in range(GA):
                xh1[b] = load_x_half(b, 1)
            for c in range(4, WP_CHUNKS):
                load_wp(c)
            brow_t = const_pool.tile([P, JL], F32)
            nc.sync.dma_start(brow_t[:], brow.ap())
            xa = [(xh0[b], xh1[b]) for b in range(GA)]

            psa = [alloc_ps(b) for b in range(GA)]
            for c in range(WP_CHUNKS):
                for b in range(GA):
                    for ko in range(wp_starts[c], wp_starts[c] + wp_sizes[c]):
                        mm(xa[b], psa[b], ko)
            for b in range(GA):
                reduce_ps(b, psa[b])

            # Final: s(p) = p * (1 + tanh(C0*(p + C1*p^3))) = 2*gelu(p)
            out_r = out.ap().rearrange("t p -> p t")

            def finals(b0, b1, part):
                n = b1 - b0
                res = []
                for acc in (mx, mn):
                    i = len(res)
                    red = fin_pool.tile([P, n], F32, tag=f"red{i}", name=f"red{i}_{part}")
                    nc.vector.tensor_reduce(
                        red[:], acc[:, b0:b1, :], axis=mybir.AxisListType.X,
                        op=mybir.AluOpType.max if acc is mx else mybir.AluOpType.min,
                    )
                    p2 = fin_pool.tile([P, n], F32, tag=f"p2_{i}", name=f"p2_{i}_{part}")
                    nc.vector.tensor_mul(p2[:], red[:], red[:])
                    p3 = fin_pool.tile([P, n], F32, tag=f"p3_{i}", name=f"p3_{i}_{part}")
                    nc.vector.tensor_mul(p3[:], p2[:], red[:])
                    w = fin_pool.tile([P, n], F32, tag=f"w_{i}", name=f"w_{i}_{part}")
                    nc.vector.scalar_tensor_tensor(
                        w[:], p3[:], C1, red[:],
                        op0=mybir.AluOpType.mult, op1=mybir.AluOpType.add,
                    )
                    th = fin_pool.tile([P, n], F32, tag=f"th_{i}", name=f"th_{i}_{part}")
                    nc.scalar.activation(
                        th[:], w[:], mybir.ActivationFunctionType.Tanh, scale=C0,
                    )
                    s = fin_pool.tile([P, n], F32, tag=f"s_{i}", name=f"s_{i}_{part}")
                    nc.vector.scalar_tensor_tensor(
                        s[:], th[:], 1.0, red[:],
                        op0=mybir.AluOpType.add, op1=mybir.AluOpType.mult,
                    )
                    res.append(s)
                o_t = fin_pool.tile([P, n], F32, tag="o", name=f"o_{part}")
                nc.vector.tensor_tensor(
                    o_t[:], res[0][:], res[1][:], op=mybir.AluOpType.max
                )
                nc.sync.dma_start(out_r[:, b0:b1], o_t[:])

            NB_HEAD = NB - 2
            for b in range(GA, NB):
                x_t = load_x(b)
                pss = alloc_ps(b)
                for ko in range(KO):
                    mm(x_t, pss, ko)
                reduce_ps(b, pss)
                if b == NB_HEAD - 1:
                    # head finals overlap the last b-tiles' matmuls
                    finals(0, NB_HEAD, "h")
            finals(NB_HEAD, NB, "t")
    nc.compile()
    return nc


def _get_module():
    global _cached
    if _cached is None:
        _cached = _build()
    return _cached


def kernel(x: np.ndarray, weight: np.ndarray, bias: np.ndarray) -> np.ndarray:
    assert x.shape == (B, IN) and weight.shape == (OUT, IN) and bias.shape == (OUT,)
    x = np.ascontiguousarray(x, dtype=np.float32)
    # Pool-fold the weights/bias (float64 accumulate, cast back).
    wp = weight.astype(np.float64).reshape(J, POOL_K, IN).mean(axis=1)
    wpT = np.ascontiguousarray(wp.T, dtype=np.float32)            # [IN, J]
    bias_p = bias.astype(np.float64).reshape(J, POOL_K).mean(axis=1).astype(np.float32)

    nc = _get_module()
    in_maps = []
    for c in range(N_CORES):
        s, t = c % BS, c // BS
        xt = np.ascontiguousarray(x[s * BL:(s + 1) * BL, :].T)    # [IN, BL]
        wpc = np.ascontiguousarray(wpT[:, t * JL:(t + 1) * JL])   # [IN, JL]
        brow = np.ascontiguousarray(
            np.broadcast_to(bias_p[t * JL:(t + 1) * JL], (P, JL))
        )
        in_maps.append({"xt": xt, "wp": wpc, "brow": brow})
    res = bass_utils.run_bass_kernel_spmd(
        nc, in_maps, core_ids=list(range(N_CORES)),
        trace=bool(os.environ.get("BASS_KERNEL_TRACE")),
    )
    global last_results
    last_results = res
    parts = [r["out"].reshape(BL) for r in res.results]
    # combine the two j-shards (max commutes with sharding), concat b-shards
    out = np.maximum(np.concatenate(parts[:BS]), np.concatenate(parts[BS:]))
    return out.astype(np.float32)


last_results = None


if __name__ == "__main__":
    rng = np.random.default_rng(0)
    x = rng.standard_normal((B, IN), dtype=np.float32)
    w = (rng.standard_normal((OUT, IN)) * (1.0 / np.sqrt(IN))).astype(np.float32)
    b = (rng.standard_normal(OUT) * 0.01).astype(np.float32)
    o = kernel(x, w, b)
    print(o.shape, o.dtype, o[:8])


# revision 33
# speedup vs baseline: 1.0262x; 1.0262x over previous
"""Fused dense_mlp kernel for TRN2 (8 NeuronCores, Bass/Tile).

reference math:
    y = x @ W.T + bias               # [B, OUT]
    pooled = avgpool_k4(y)           # [B, OUT/4]
    out = max_j( 2 * gelu_tanh(pooled) )   # [B]

Algebraic restructuring (exact, up to fp rounding):
  * avg-pool commutes with the linear layer:
        pooled = x @ Wp.T + bias_p,  Wp = mean of each 4-row group of W
    -> the GEMM shrinks 4x to [B, K] @ [K, J], K=4096, J=2048.
  * gelu_tanh is quasiconvex (single minimum ~ -0.75), so
        max_j gelu(p_j) = max(gelu(row_max), gelu(row_min))
    -> only 2 gelu evaluations per row, after cheap row max/min reductions.
  * SCALE=2 cancels gelu's 0.5:  2*gelu(p) = p * (1 + tanh(c0*(p + c1*p^3))).
  * the j-max commutes with sharding j: each core reports its partial
    max over its j-range; the host combines with an elementwise max.

Distribution: 2D sharding - 4 batch shards x 2 j shards. Core (t*4+s)
handles rows [s*4096,(s+1)*4096) and pooled features [t*1024,(t+1)*1024).
Its Wp half (16.8 MB) is fully SBUF-resident, x streams through exactly
once (k-major, pre-transposed on host).

Matmuls run as float32r (TF32-like fast fp32 path, 1 row/cycle at N=512).
Each stationary x tile feeds 2 PSUM banks (the two 512-wide j tiles), so
LDWEIGHTS is amortized over 2 matmuls. Bias is accumulated into PSUM via
one extra matmul with a ones-row stationary operand.
"""

import os
import sys

for _p in ("/opt/trn_rl_repo",):
    if _p not in sys.path:
        sys.path.append(_p)

import numpy as np

import concourse.bass as bass
import concourse.mybir as mybir
import concourse.tile as tile
from concourse import bacc, bass_utils

# Problem shapes (hardcoded per contract).
B, IN, OUT = 16384, 4096, 8192
POOL_K = 4
J = OUT // POOL_K            # 2048 pooled features
N_CORES = 8
BS = 4                       # batch shards
JS = 2                       # j shards
BL = B // BS                 # 4096 batch rows per core
JL = J // JS                 # 1024 pooled features per core
P = 128                      # partitions
KO = IN // P                 # 32 k-subtiles
NB = BL // P                 # 32 b-tiles per core
JT = 512                     # j-tile width (one PSUM bank)
NJ = JL // JT                # 2 j-tiles per core
WP_CHUNKS = 8                # split the resident wp load to unblock early MMs

C0 = 0.7978845608            # sqrt(2/pi) as used by the reference
C1 = 0.044715

F32 = mybir.dt.float32
F32R = mybir.dt.float32r

_cached = None


def _build():
    nc = bacc.Bacc("TRN2", target_bir_lowering=False)
    xt = nc.dram_tensor("xt", [IN, BL], F32R, kind="ExternalInput")
    wp = nc.dram_tensor("wp", [IN, JL], F32R, kind="ExternalInput")
    brow = nc.dram_tensor("brow", [P, JL], F32, kind="ExternalInput")
    out = nc.dram_tensor("out", [NB, P], F32, kind="ExternalOutput")

    xt_r = xt.ap().rearrange("(ko ki) b -> ki ko b", ki=P)
    wp_r = wp.ap().rearrange("(ko ki) j -> ki ko j", ki=P)

    with tile.TileContext(nc) as tc:
        with (
            tc.tile_pool(name="wpp", bufs=1) as wp_pool,
            tc.tile_pool(name="xp", bufs=6) as x_pool,
            tc.tile_pool(name="cst", bufs=1) as const_pool,
            tc.tile_pool(name="red", bufs=1) as red_pool,
            tc.tile_pool(name="fin", bufs=1) as fin_pool,
            tc.tile_pool(name="psum", bufs=4, space="PSUM") as psum_pool,
        ):
            # Whole wp half resident, as separate chunk tiles so the PE can
            # start as soon as chunk 0 lands (precise per-chunk deps).
            # First chunks are small so the first matmul starts early.
            wp_sizes = [4, 4, 4, 4, 4, 4, 4, 4]
            assert sum(wp_sizes) == KO and len(wp_sizes) == WP_CHUNKS
            wp_starts = [sum(wp_sizes[:c]) for c in range(WP_CHUNKS)]
            wp_ts = [None] * WP_CHUNKS
            ko2chunk = []
            for c, sz in enumerate(wp_sizes):
                ko2chunk += [c] * sz

            def load_wp(c):
                wpc_t = wp_pool.tile(
                    [P, wp_sizes[c], JL], F32R, tag=f"wp{c}", name=f"wp{c}"
                )
                nc.sync.dma_start(
                    wpc_t[:], wp_r[:, wp_starts[c]:wp_starts[c] + wp_sizes[c], :]
                )
                wp_ts[c] = wpc_t

            mx = red_pool.tile([P, NB, NJ], F32)
            mn = red_pool.tile([P, NB, NJ], F32)

            BIG = 3.0e38

            KH = KO // 2

            def load_x_half(b, h):
                t = x_pool.tile([P, KH, P], F32R, tag="x", name=f"x_{b}h{h}")
                nc.sync.dma_start(
                    t[:], xt_r[:, h * KH:(h + 1) * KH, b * P:(b + 1) * P]
                )
                return t

            def load_x(b):
                # two half tiles: finer slot release -> deeper x prefetch
                return (load_x_half(b, 0), load_x_half(b, 1))

            def alloc_ps(b):
                return [psum_pool.tile([P, JT], F32, tag=f"ps{j}", name=f"ps{j}_{b}")
                        for j in range(NJ)]

            def mm(x_pair, pss, ko):
                c = ko2chunk[ko]
                x_t = x_pair[ko // KH]
                for j, ps in enumerate(pss):
                    nc.tensor.matmul(
                        ps[:], lhsT=x_t[:, ko % KH, :],
                        rhs=wp_ts[c][:, ko - wp_starts[c], j * JT:(j + 1) * JT],
                        start=(ko == 0), stop=(ko == KO - 1),
                    )

            def reduce_ps(b, pss):
                # bias-add in place (DVE), then row max / min reductions
                for j, ps in enumerate(pss):
                    nc.vector.scalar_tensor_tensor(
                        ps[:], ps[:], 0.0, brow_t[:, j * JT:(j + 1) * JT],
                        op0=mybir.AluOpType.bypass, op1=mybir.AluOpType.add,
                    )
                    nc.vector.tensor_reduce(
                        mx[:, b, j:j + 1], ps[:], axis=mybir.AxisListType.X,
                        op=mybir.AluOpType.max, opt_output=False,
                    )
                    nc.vector.tensor_reduce(
                        mn[:, b, j:j + 1], ps[:], axis=mybir.AxisListType.X,
                        op=mybir.AluOpType.min, opt_output=False,
                    )

            # Warmup group: first GA b-tiles run chunk-major so the PE has
            # work while the later wp chunks are still loading. DMA issue
            # order interleaves the first x tiles with the wp chunks so the
            # first matmul can start after ~4 MB of traffic, and brow (only
            # needed by the first reduce) goes last.
            GA = 3
            xa = [None] * GA
            xa[0] = load_x(0)
            load_wp(0)
            xa[1] = load_x(1)
            load_wp(1)
            xa[2] = load_x(2)
            for c in range(2, WP_CHUNKS):
                load_wp(c)
            brow_t = const_pool.tile([P, JL], F32)
            nc.sync.dma_start(brow_t[:], brow.ap())

            psa = [alloc_ps(b) for b in range(GA)]
            for c in range(WP_CHUNKS):
                for b in range(GA):
                    for ko in range(wp_starts[c], wp_starts[c] + wp_sizes[c]):
                        mm(xa[b], psa[b], ko)
            for b in range(GA):
                reduce_ps(b, psa[b])

            # Final: s(p) = p * (1 + tanh(C0*(p + C1*p^3))) = 2*gelu(p)
            out_r = out.ap().rearrange("t p -> p t")

            def finals(b0, b1, part):
                n = b1 - b0
                res = []
                for acc in (mx, mn):
                    i = len(res)
                    red = fin_pool.tile([P, n], F32, tag=f"red{i}", name=f"red{i}_{part}")
                    nc.vector.tensor_reduce(
                        red[:], acc[:, b0:b1, :], axis=mybir.AxisListType.X,
                        op=mybir.AluOpType.max if acc is mx else mybir.AluOpType.min,
                    )
                    p2 = fin_pool.tile([P, n], F32, tag=f"p2_{i}", name=f"p2_{i}_{part}")
                    nc.vector.tensor_mul(p2[:], red[:], red[:])
                    p3 = fin_pool.tile([P, n], F32, tag=f"p3_{i}", name=f"p3_{i}_{part}")
                    nc.vector.tensor_mul(p3[:], p2[:], red[:])
                    w = fin_pool.tile([P, n], F32, tag=f"w_{i}", name=f"w_{i}_{part}")
                    nc.vector.scalar_tensor_tensor(
                        w[:], p3[:], C1, red[:],
                        op0=mybir.AluOpType.mult, op1=mybir.AluOpType.add,
                    )
                    th = fin_pool.tile([P, n], F32, tag=f"th_{i}", name=f"th_{i}_{part}")
                    nc.scalar.activation(
                        th[:], w[:], mybir.ActivationFunctionType.Tanh, scale=C0,
                    )
                    s = fin_pool.tile([P, n], F32, tag=f"s_{i}", name=f"s_{i}_{part}")
                    nc.vector.scalar_tensor_tensor(
                        s[:], th[:], 1.0, red[:],
                        op0=mybir.AluOpType.add, op1=mybir.AluOpType.mult,
                    )
                    res.append(s)
                o_t = fin_pool.tile([P, n], F32, tag="o", name=f"o_{part}")
                nc.vector.tensor_tensor(
                    o_t[:], res[0][:], res[1][:], op=mybir.AluOpType.max
                )
                nc.sync.dma_start(out_r[:, b0:b1], o_t[:])

            NB_HEAD = NB - 2
            for b in range(GA, NB):
                x_t = load_x(b)
                pss = alloc_ps(b)
                for ko in range(KO):
                    mm(x_t, pss, ko)
                reduce_ps(b, pss)
                if b == NB_HEAD - 1:
                    # head finals overlap the last b-tiles' matmuls
                    finals(0, NB_HEAD, "h")
            finals(NB_HEAD, NB, "t")
    nc.compile()
    return nc


def _get_module():
    global _cached
    if _cached is None:
        _cached = _build()
    return _cached


def kernel(x: np.ndarray, weight: np.ndarray, bias: np.ndarray) -> np.ndarray:
    assert x.shape == (B, IN) and weight.shape == (OUT, IN) and bias.shape == (OUT,)
    x = np.ascontiguousarray(x, dtype=np.float32)
    # Pool-fold the weights/bias (float64 accumulate, cast back).
    wp = weight.astype(np.float64).reshape(J, POOL_K, IN).mean(axis=1)
    wpT = np.ascontiguousarray(wp.T, dtype=np.float32)            # [IN, J]
    bias_p = bias.astype(np.float64).reshape(J, POOL_K).mean(axis=1).astype(np.float32)

    nc = _get_module()
    in_maps = []
    for c in range(N_CORES):
        s, t = c % BS, c // BS
        xt = np.ascontiguousarray(x[s * BL:(s + 1) * BL, :].T)    # [IN, BL]
        wpc = np.ascontiguousarray(wpT[:, t * JL:(t + 1) * JL])   # [IN, JL]
        brow = np.ascontiguousarray(
            np.broadcast_to(bias_p[t * JL:(t + 1) * JL], (P, JL))
        )
        in_maps.append({"xt": xt, "wp": wpc, "brow": brow})
    res = bass_utils.run_bass_kernel_spmd(
        nc, in_maps, core_ids=list(range(N_CORES)),
        trace=bool(os.environ.get("BASS_KERNEL_TRACE")),
    )
    global last_results
    last_results = res
    parts = [r["out"].reshape(BL) for r in res.results]
    # combine the two j-shards (max commutes with sharding), concat b-shards
    out = np.maximum(np.concatenate(parts[:BS]), np.concatenate(parts[BS:]))
    return out.astype(np.float32)


last_results = None


if __name__ == "__main__":
    rng = np.random.default_rng(0)
    x = rng.standard_normal((B, IN), dtype=np.float32)
    w = (rng.standard_normal((OUT, IN)) * (1.0 / np.sqrt(IN))).astype(np.float32)
    b = (rng.standard_normal(OUT) * 0.01).astype(np.float32)
    o = kernel(x, w, b)
    print(o.shape, o.dtype, o[:8])


# revision 35
# speedup vs baseline: 1.0466x; 1.0199x over previous
"""Fused dense_mlp kernel for TRN2 (8 NeuronCores, Bass/Tile).

reference math:
    y = x @ W.T + bias               # [B, OUT]
    pooled = avgpool_k4(y)           # [B, OUT/4]
    out = max_j( 2 * gelu_tanh(pooled) )   # [B]

Algebraic restructuring (exact, up to fp rounding):
  * avg-pool commutes with the linear layer:
        pooled = x @ Wp.T + bias_p,  Wp = mean of each 4-row group of W
    -> the GEMM shrinks 4x to [B, K] @ [K, J], K=4096, J=2048.
  * gelu_tanh is quasiconvex (single minimum ~ -0.75), so
        max_j gelu(p_j) = max(gelu(row_max), gelu(row_min))
    -> only 2 gelu evaluations per row, after cheap row max/min reductions.
  * SCALE=2 cancels gelu's 0.5:  2*gelu(p) = p * (1 + tanh(c0*(p + c1*p^3))).
  * the j-max commutes with sharding j: each core reports its partial
    max over its j-range; the host combines with an elementwise max.

Distribution: 2D sharding - 4 batch shards x 2 j shards. Core (t*4+s)
handles rows [s*4096,(s+1)*4096) and pooled features [t*1024,(t+1)*1024).
Its Wp half (16.8 MB) is fully SBUF-resident, x streams through exactly
once (k-major, pre-transposed on host).

Matmuls run as float32r (TF32-like fast fp32 path, 1 row/cycle at N=512).
Each stationary x tile feeds 2 PSUM banks (the two 512-wide j tiles), so
LDWEIGHTS is amortized over 2 matmuls. Bias is accumulated into PSUM via
one extra matmul with a ones-row stationary operand.
"""

import os
import sys

for _p in ("/opt/trn_rl_repo",):
    if _p not in sys.path:
        sys.path.append(_p)

import numpy as np

import concourse.bass as bass
import concourse.mybir as mybir
import concourse.tile as tile
from concourse import bacc, bass_utils

# Problem shapes (hardcoded per contract).
B, IN, OUT = 16384, 4096, 8192
POOL_K = 4
J = OUT // POOL_K            # 2048 pooled features
N_CORES = 8
BS = 4                       # batch shards
JS = 2                       # j shards
BL = B // BS                 # 4096 batch rows per core
JL = J // JS                 # 1024 pooled features per core
P = 128                      # partitions
KO = IN // P                 # 32 k-subtiles
NB = BL // P                 # 32 b-tiles per core
JT = 512                     # j-tile width (one PSUM bank)
NJ = JL // JT                # 2 j-tiles per core
WP_CHUNKS = 8                # split the resident wp load to unblock early MMs

C0 = 0.7978845608            # sqrt(2/pi) as used by the reference
C1 = 0.044715

F32 = mybir.dt.float32
F32R = mybir.dt.float32r

_cached = None


def _build():
    nc = bacc.Bacc("TRN2", target_bir_lowering=False)
    xt = nc.dram_tensor("xt", [IN, BL], F32R, kind="ExternalInput")
    wp = nc.dram_tensor("wp", [IN, JL], F32R, kind="ExternalInput")
    brow = nc.dram_tensor("brow", [P, JL], F32, kind="ExternalInput")
    out = nc.dram_tensor("out", [NB, P], F32, kind="ExternalOutput")

    xt_r = xt.ap().rearrange("(ko ki) b -> ki ko b", ki=P)
    wp_r = wp.ap().rearrange("(ko ki) j -> ki ko j", ki=P)

    with tile.TileContext(nc) as tc:
        with (
            tc.tile_pool(name="wpp", bufs=1) as wp_pool,
            tc.tile_pool(name="xp", bufs=6) as x_pool,
            tc.tile_pool(name="cst", bufs=1) as const_pool,
            tc.tile_pool(name="red", bufs=1) as red_pool,
            tc.tile_pool(name="fin", bufs=1) as fin_pool,
            tc.tile_pool(name="psum", bufs=4, space="PSUM") as psum_pool,
        ):
            # Whole wp half resident, as separate chunk tiles so the PE can
            # start as soon as chunk 0 lands (precise per-chunk deps).
            wp_sizes = [KO // WP_CHUNKS] * WP_CHUNKS
            assert sum(wp_sizes) == KO and len(wp_sizes) == WP_CHUNKS
            wp_starts = [sum(wp_sizes[:c]) for c in range(WP_CHUNKS)]
            wp_ts = [None] * WP_CHUNKS
            ko2chunk = []
            for c, sz in enumerate(wp_sizes):
                ko2chunk += [c] * sz

            def load_wp(c):
                wpc_t = wp_pool.tile(
                    [P, wp_sizes[c], JL], F32R, tag=f"wp{c}", name=f"wp{c}"
                )
                nc.sync.dma_start(
                    wpc_t[:], wp_r[:, wp_starts[c]:wp_starts[c] + wp_sizes[c], :]
                )
                wp_ts[c] = wpc_t

            mx = red_pool.tile([P, NB, NJ], F32)
            mn = red_pool.tile([P, NB, NJ], F32)

            KH = KO // 2

            def load_x_half(b, h):
                t = x_pool.tile([P, KH, P], F32R, tag="x", name=f"x_{b}h{h}")
                nc.sync.dma_start(
                    t[:], xt_r[:, h * KH:(h + 1) * KH, b * P:(b + 1) * P]
                )
                return t

            def load_x(b):
                # two half tiles: finer slot release -> deeper x prefetch
                return (load_x_half(b, 0), load_x_half(b, 1))

            def alloc_ps(b):
                return [psum_pool.tile([P, JT], F32, tag=f"ps{j}", name=f"ps{j}_{b}")
                        for j in range(NJ)]

            def mm(x_pair, pss, ko):
                c = ko2chunk[ko]
                x_t = x_pair[ko // KH]
                for j, ps in enumerate(pss):
                    nc.tensor.matmul(
                        ps[:], lhsT=x_t[:, ko % KH, :],
                        rhs=wp_ts[c][:, ko - wp_starts[c], j * JT:(j + 1) * JT],
                        start=(ko == 0), stop=(ko == KO - 1),
                    )

            def reduce_ps(b, pss):
                # bias-add in place (DVE), then row max / min reductions
                for j, ps in enumerate(pss):
                    nc.vector.scalar_tensor_tensor(
                        ps[:], ps[:], 0.0, brow_t[:, j * JT:(j + 1) * JT],
                        op0=mybir.AluOpType.bypass, op1=mybir.AluOpType.add,
                    )
                    nc.vector.tensor_reduce(
                        mx[:, b, j:j + 1], ps[:], axis=mybir.AxisListType.X,
                        op=mybir.AluOpType.max, opt_output=False,
                    )
                    nc.vector.tensor_reduce(
                        mn[:, b, j:j + 1], ps[:], axis=mybir.AxisListType.X,
                        op=mybir.AluOpType.min, opt_output=False,
                    )

            # Warmup group: first GA b-tiles run chunk-major so the PE has
            # work while the later wp chunks are still loading. DMA issue
            # order interleaves the first x tiles with the wp chunks so the
            # first matmul can start after ~4 MB of traffic, and brow (only
            # needed by the first reduce) goes last.
            GA = 3
            xa = [None] * GA
            xa[0] = load_x(0)
            load_wp(0)
            xa[1] = load_x(1)
            load_wp(1)
            xa[2] = load_x(2)
            for c in range(2, WP_CHUNKS):
                load_wp(c)
            brow_t = const_pool.tile([P, JL], F32)
            nc.sync.dma_start(brow_t[:], brow.ap())

            psa = [alloc_ps(b) for b in range(GA)]
            for c in range(WP_CHUNKS):
                for b in range(GA):
                    for ko in range(wp_starts[c], wp_starts[c] + wp_sizes[c]):
                        mm(xa[b], psa[b], ko)
            for b in range(GA):
                reduce_ps(b, psa[b])

            # Final: s(p) = p * (1 + tanh(C0*(p + C1*p^3))) = 2*gelu(p)
            out_r = out.ap().rearrange("t p -> p t")

            def finals(b0, b1, part):
                n = b1 - b0
                res = []
                for acc in (mx, mn):
                    i = len(res)
                    red = fin_pool.tile([P, n], F32, tag=f"red{i}", name=f"red{i}_{part}")
                    nc.vector.tensor_reduce(
                        red[:], acc[:, b0:b1, :], axis=mybir.AxisListType.X,
                        op=mybir.AluOpType.max if acc is mx else mybir.AluOpType.min,
                    )
                    p2 = fin_pool.tile([P, n], F32, tag=f"p2_{i}", name=f"p2_{i}_{part}")
                    nc.vector.tensor_mul(p2[:], red[:], red[:])
                    p3 = fin_pool.tile([P, n], F32, tag=f"p3_{i}", name=f"p3_{i}_{part}")
                    nc.vector.tensor_mul(p3[:], p2[:], red[:])
                    w = fin_pool.tile([P, n], F32, tag=f"w_{i}", name=f"w_{i}_{part}")
                    nc.vector.scalar_tensor_tensor(
                        w[:], p3[:], C1, red[:],
                        op0=mybir.AluOpType.mult, op1=mybir.AluOpType.add,
                    )
                    th = fin_pool.tile([P, n], F32, tag=f"th_{i}", name=f"th_{i}_{part}")
                    nc.scalar.activation(
                        th[:], w[:], mybir.ActivationFunctionType.Tanh, scale=C0,
                    )
                    s = fin_pool.tile([P, n], F32, tag=f"s_{i}", name=f"s_{i}_{part}")
                    nc.vector.scalar_tensor_tensor(
                        s[:], th[:], 1.0, red[:],
                        op0=mybir.AluOpType.add, op1=mybir.AluOpType.mult,
                    )
                    res.append(s)
                o_t = fin_pool.tile([P, n], F32, tag="o", name=f"o_{part}")
                nc.vector.tensor_tensor(
                    o_t[:], res[0][:], res[1][:], op=mybir.AluOpType.max
                )
                nc.sync.dma_start(out_r[:, b0:b1], o_t[:])

            NB_HEAD = NB - 2
            for b in range(GA, NB):
                x_t = load_x(b)
                pss = alloc_ps(b)
                for ko in range(KO):
                    mm(x_t, pss, ko)
                reduce_ps(b, pss)
                if b == NB_HEAD - 1:
                    # head finals overlap the last b-tiles' matmuls
                    finals(0, NB_HEAD, "h")
            finals(NB_HEAD, NB, "t")
    nc.compile()
    return nc


def _get_module():
    global _cached
    if _cached is None:
        _cached = _build()
    return _cached


def kernel(x: np.ndarray, weight: np.ndarray, bias: np.ndarray) -> np.ndarray:
    assert x.shape == (B, IN) and weight.shape == (OUT, IN) and bias.shape == (OUT,)
    x = np.ascontiguousarray(x, dtype=np.float32)
    # Pool-fold the weights/bias (float64 accumulate, cast back).
    wp = weight.astype(np.float64).reshape(J, POOL_K, IN).mean(axis=1)
    wpT = np.ascontiguousarray(wp.T, dtype=np.float32)            # [IN, J]
    bias_p = bias.astype(np.float64).reshape(J, POOL_K).mean(axis=1).astype(np.float32)

    nc = _get_module()
    in_maps = []
    for c in range(N_CORES):
        s, t = c % BS, c // BS
        xt = np.ascontiguousarray(x[s * BL:(s + 1) * BL, :].T)    # [IN, BL]
        wpc = np.ascontiguousarray(wpT[:, t * JL:(t + 1) * JL])   # [IN, JL]
        brow = np.ascontiguousarray(
            np.broadcast_to(bias_p[t * JL:(t + 1) * JL], (P, JL))
        )
        in_maps.append({"xt": xt, "wp": wpc, "brow": brow})
    res = bass_utils.run_bass_kernel_spmd(
        nc, in_maps, core_ids=list(range(N_CORES)),
        trace=bool(os.environ.get("BASS_KERNEL_TRACE")),
    )
    global last_results
    last_results = res
    parts = [r["out"].reshape(BL) for r in res.results]
    # combine the two j-shards (max commutes with sharding), concat b-shards
    out = np.maximum(np.concatenate(parts[:BS]), np.concatenate(parts[BS:]))
    return out.astype(np.float32)


last_results = None


if __name__ == "__main__":
    rng = np.random.default_rng(0)
    x = rng.standard_normal((B, IN), dtype=np.float32)
    w = (rng.standard_normal((OUT, IN)) * (1.0 / np.sqrt(IN))).astype(np.float32)
    b = (rng.standard_normal(OUT) * 0.01).astype(np.float32)
    o = kernel(x, w, b)
    print(o.shape, o.dtype, o[:8])
